# revision 13
# baseline (speedup 1.0000x reference)
"""Sharded top-1 KNN (retrieval) on 8 TRN2 NeuronCores via Bass/Tile.

v3 strategy (hardcoded for x[2048,24,16], X_train[65536,384], Y_train[65536,24,1]):
  - Shard X_train rows across 8 cores (8192 rows each).
  - fp8(e4m3) DoubleRow matmuls: K padded 384->512, packed 2 rows per
    partition, so each [128q, 512n] PSUM chunk takes two K=256 matmuls at
    0.5 cycles/row (4x bf16 row rate).
  - Per core the shard rows are permuted so every pooled output column
    mixes 4 rows that are adjacent in ||t||^2 order.  The 16 PSUM chunks of
    a query tile are max-folded 4->1 by a drain spread across Pool / DVE /
    ACT (leaf pair-maxes + 2-wide fp16 merges), giving 4 pooled tiles
    [128,512] per m-tile, shipped to HBM as fp16 (2048 pooled cols/core).
  - Host: applies the shared -||t||^2/2 bias per pooled column, takes the
    global top-K pooled columns per query (K=32 across 8*2048=16384 cols),
    expands 4 rows per column, recomputes exact distances in float64 for
    the <=128 candidates, argmin (ties: smallest global index, matching
    jnp.argmin), returns Y_train[best].
  Max-pooling cannot hurt candidate recall: the true NN's pooled column
  value >= its own score; empirically (seed-0 data, fp8+fp16 emulation)
  the NN's pooled column ranks <=9 globally, vs K=32 kept.
"""

import os
import sys

import numpy as np

for _p in ("/opt/trn_rl_repo",):
    if os.path.isdir(_p) and _p not in sys.path:
        sys.path.insert(0, _p)

import ml_dtypes  # noqa: E402

B, T, F = 2048, 24, 16
D = T * F  # 384
DPAD = 512
N = 65536
NCORES = 8
NS = N // NCORES  # 8192 rows per core
MT = B // 128  # 16 query tiles
NCHUNK = 512
NT = NS // NCHUNK  # 16 train chunks per core
FOLD = 4  # quad structure for the tt-adjacent permutation
NVT = NT // FOLD  # 4 quads per m-tile
NF2 = NS // 2  # 4096 fold2 pooled columns per core
TOPK = 48  # pooled columns kept per query (host-side, global)

_FP8 = ml_dtypes.float8_e4m3


def build_nc(b=B, ns=NS):
    """Build the per-core Bass program (SPMD: same program, per-core inputs)."""
    import concourse.tile as tile
    from concourse import bacc, mybir

    mt = b // 128
    nt = ns // NCHUNK

    nc = bacc.Bacc(None, target_bir_lowering=False)
    # [p, m, t, i, q] = xTpad[256t+128i+p, 128m+q]  (fp8, K padded to 512)
    xT8 = nc.dram_tensor(
        "xT8", [128, mt, 2, 2, 128], mybir.dt.float8e4, kind="ExternalInput"
    )
    # [p, c, t, i, n] = XTpad[256t+128i+p, 512c+n]
    XT8 = nc.dram_tensor(
        "XT8", [128, nt, 2, 2, NCHUNK], mybir.dt.float8e4, kind="ExternalInput"
    )
    # pooled[q, q4*1024 + s*512 + n] = fold2 of chunks {4*q4+s, 4*q4+2+s}
    pooled = nc.dram_tensor(
        "pooled", [b, NVT * 2 * NCHUNK], mybir.dt.float16, kind="ExternalOutput"
    )

    MAX = mybir.AluOpType.max

    # TRN2 drain rules: GPSIMD can't touch PSUM; any op reads at most ONE
    # non-scalar PSUM operand (a single AP may still span 2 banks).  So per
    # quad of chunks {4q..4q+3}: ACT 2-wide-copies the even pair to SBUF
    # fp16, DVE folds the odd pair's 2-bank PSUM AP against the copy
    # (tensor_tensor max), and Pool does the all-SBUF 2-wide finals.

    with tile.TileContext(nc) as tc:
        with (
            tc.tile_pool(name="wpool", bufs=1) as wpool,
            tc.tile_pool(name="ppool", bufs=4, space="PSUM") as ppool,
            tc.tile_pool(name="lpool", bufs=2) as lpool,
            tc.tile_pool(name="opool", bufs=3) as opool,
        ):
            # Per-chunk tiles so each matmul waits only on ITS chunk's DMA
            # (subtile deps on one big tile made the first matmul wait for the
            # whole 4.2MB input stream).  Loads split across the SP and GpSimd
            # DGE queues so per-dma issue cost doesn't serialize the startup.
            xT_m0 = wpool.tile([128, 2, 2, 128], mybir.dt.float8e4, name="xT_m0")
            xT_r = wpool.tile([128, mt - 1, 2, 2, 128], mybir.dt.float8e4, name="xT_r")
            nc.sync.dma_start(xT_m0[:], xT8[:, 0])
            nc.gpsimd.dma_start(out=xT_r[:], in_=xT8[:, 1:])
            XT_c = []
            for c in range(nt):
                xtc = wpool.tile(
                    [128, 2, 2, NCHUNK], mybir.dt.float8e4, name="xtc", tag=f"xt{c}"
                )
                XT_c.append(xtc)
                if c % 2 == 0:
                    nc.sync.dma_start(xtc[:], XT8[:, c])
                else:
                    nc.gpsimd.dma_start(out=xtc[:], in_=XT8[:, c])

            def xT_ap(m, t):
                return xT_m0[:, t] if m == 0 else xT_r[:, m - 1, t]

            for m in range(mt):
                # F[p, q, s, n]: fold2 of chunks {4q+s, 4q+2+s}; shipped as-is
                F = lpool.tile([128, NVT, 2, NCHUNK], mybir.dt.float16, name="F")
                for h in range(2):  # half h = pairs 4h..4h+3 = 8 psum banks
                    pts = [
                        ppool.tile(
                            [128, 2, NCHUNK], mybir.dt.float32, name="pt", tag="pt"
                        )
                        for _ in range(4)
                    ]
                    # t-outer: 8 consecutive matmuls share the stationary
                    # operand; the hoisted ldweights lets walrus pair it with
                    # non-self-loading matmuls.
                    for t in range(2):
                        nc.tensor.ldweights(
                            xT_ap(m, t), perf_mode=mybir.MatmulPerfMode.DoubleRow
                        )
                        for pr in range(4):
                            for s in range(2):
                                nc.tensor.matmul(
                                    pts[pr][:, s],
                                    xT_ap(m, t),
                                    XT_c[8 * h + 2 * pr + s][:, t],
                                    start=(t == 0),
                                    stop=(t == 1),
                                    perf_mode=mybir.MatmulPerfMode.DoubleRow,
                                )
                    for j in range(2):  # quad q = 2h+j: pairs (2j, 2j+1) of half
                        q = 2 * h + j
                        c2 = lpool.tile(
                            [128, 2, NCHUNK],
                            mybir.dt.float16,
                            name="c2",
                            tag=f"c2{j}",
                        )
                        nc.scalar.copy(c2[:], pts[2 * j][:])
                        nc.vector.tensor_tensor(F[:, q], pts[2 * j + 1][:], c2[:], op=MAX)
                nc.sync.dma_start(
                    pooled[m * 128 : (m + 1) * 128, :],
                    F.rearrange("p q s n -> p (q s n)"),
                )
    nc.finalize()
    return nc


_NC = None


def _get_nc():
    global _NC
    if _NC is None:
        _NC = build_nc()
    return _NC


def _shard_perm(tt, ns):
    """Device row (FOLD*v+u)*512+n holds tt-sorted rank (v*512+n)*FOLD+u, so
    the 4 rows folded into pooled column (v,n) are adjacent in tt order."""
    order = np.argsort(tt, kind="stable")
    r = np.arange(ns)
    j, col = r // NCHUNK, r % NCHUNK
    v, u = j // FOLD, j % FOLD
    return order[(v * NCHUNK + col) * FOLD + u]


def _prep_in_maps(xf, X_train):
    xTpad = np.zeros((DPAD, B), np.float32)
    xTpad[:D] = xf.T
    xT8 = np.ascontiguousarray(
        xTpad.astype(_FP8).reshape(2, 2, 128, MT, 128).transpose(2, 3, 0, 1, 4)
    )
    in_maps = []
    perms = []
    biases = []
    for c in range(NCORES):
        Xs = X_train[c * NS : (c + 1) * NS]
        tt = (Xs.astype(np.float64) ** 2).sum(axis=1)
        perm = _shard_perm(tt, NS)
        perms.append(perm)
        XTpad = np.zeros((DPAD, NS), np.float32)
        XTpad[:D] = Xs[perm].T
        XT8 = np.ascontiguousarray(
            XTpad.astype(_FP8).reshape(2, 2, 128, NT, NCHUNK).transpose(2, 3, 0, 1, 4)
        )
        tt_dev = tt[perm]
        # pooled col (q*1024 + s*512 + n) folds device rows
        # {(4q+s)*512+n, (4q+2+s)*512+n}; bias = mean tt/2 of the two
        td = tt_dev.reshape(NVT, 2, 2, NCHUNK)  # [q, k, s, n]
        bias = td.mean(axis=1).reshape(NF2) * 0.5  # [q, s, n] -> flat
        biases.append(bias.astype(np.float32))
        in_maps.append({"xT8": xT8, "XT8": XT8})
    return in_maps, perms, biases


def _refine(xf, X_train, Y_train, cand):
    """cand: [B, C] global candidate row indices (int64, may repeat)."""
    b = cand.shape[0]
    cand = np.sort(cand, axis=1)
    best = np.empty(b, dtype=np.int64)
    xd = xf.astype(np.float64)
    step = 128
    for s in range(0, b, step):
        e = min(s + step, b)
        Xc = X_train[cand[s:e]].astype(np.float64)  # [q, C, D]
        diff = xd[s:e, None, :] - Xc
        d2 = np.einsum("qcd,qcd->qc", diff, diff)
        best[s:e] = cand[s:e][np.arange(e - s), np.argmin(d2, axis=1)]
    return Y_train[best].astype(np.float32)


def kernel(x, X_train, Y_train, _trace=False, _tmpdir=None):
    from concourse.bass_utils import run_bass_kernel_spmd

    x = np.asarray(x, dtype=np.float32)
    X_train = np.asarray(X_train, dtype=np.float32)
    Y_train = np.asarray(Y_train, dtype=np.float32)
    xf = x.reshape(B, D)

    in_maps, perms, biases = _prep_in_maps(xf, X_train)
    nc = _get_nc()
    kw = {}
    if _trace:
        kw = {"trace": True, "tmpdir": _tmpdir}
    res = run_bass_kernel_spmd(nc, in_maps, core_ids=list(range(NCORES)), **kw)

    # scores[q, c*NF2 + col] = pooled - bias
    scores = np.empty((B, NCORES * NF2), np.float32)
    for c in range(NCORES):
        scores[:, c * NF2 : (c + 1) * NF2] = (
            res.results[c]["pooled"].astype(np.float32) - biases[c][None, :]
        )
    top = np.argpartition(-scores, TOPK, axis=1)[:, :TOPK]  # [B, K] pooled cols
    core = top // NF2
    col = top % NF2
    q4, rem = col // (2 * NCHUNK), col % (2 * NCHUNK)
    s, n = rem // NCHUNK, rem % NCHUNK
    # pooled col (q4,s,n) covers device rows (4q4+s)*512+n and (4q4+2+s)*512+n
    base = (4 * q4 + s) * NCHUNK + n
    devrows = np.stack([base, base + 2 * NCHUNK], axis=2)  # [B, K, 2]
    permtab = np.stack(perms)  # [NCORES, NS]
    cand = permtab[core[:, :, None], devrows] + core[:, :, None] * NS
    cand = cand.reshape(B, TOPK * 2)
    out = _refine(xf, X_train, Y_train, cand)
    if _trace:
        return out, res
    return out


# revision 19
# speedup vs baseline: 1.0887x; 1.0887x over previous
"""Sharded top-1 KNN (retrieval) on 8 TRN2 NeuronCores via Bass/Tile.

v3 strategy (hardcoded for x[2048,24,16], X_train[65536,384], Y_train[65536,24,1]):
  - Shard X_train rows across 8 cores (8192 rows each).
  - fp8(e4m3) DoubleRow matmuls: K padded 384->512, packed 2 rows per
    partition, so each [128q, 512n] PSUM chunk takes two K=256 matmuls at
    0.5 cycles/row (4x bf16 row rate).
  - Per core the shard rows are permuted so every pooled output column
    mixes 4 rows that are adjacent in ||t||^2 order.  The 16 PSUM chunks of
    a query tile are max-folded 4->1 by a drain spread across Pool / DVE /
    ACT (leaf pair-maxes + 2-wide fp16 merges), giving 4 pooled tiles
    [128,512] per m-tile, shipped to HBM as fp16 (2048 pooled cols/core).
  - Host: applies the shared -||t||^2/2 bias per pooled column, takes the
    global top-K pooled columns per query (K=32 across 8*2048=16384 cols),
    expands 4 rows per column, recomputes exact distances in float64 for
    the <=128 candidates, argmin (ties: smallest global index, matching
    jnp.argmin), returns Y_train[best].
  Max-pooling cannot hurt candidate recall: the true NN's pooled column
  value >= its own score; empirically (seed-0 data, fp8+fp16 emulation)
  the NN's pooled column ranks <=9 globally, vs K=32 kept.
"""

import os
import sys

import numpy as np

for _p in ("/opt/trn_rl_repo",):
    if os.path.isdir(_p) and _p not in sys.path:
        sys.path.insert(0, _p)

import ml_dtypes  # noqa: E402

B, T, F = 2048, 24, 16
D = T * F  # 384
DPAD = 512
N = 65536
NCORES = 8
NS = N // NCORES  # 8192 rows per core
MT = B // 128  # 16 query tiles
NCHUNK = 512
NT = NS // NCHUNK  # 16 train chunks per core
FOLD = 4  # quad structure for the tt-adjacent permutation
NVT = NT // FOLD  # 4 quads per m-tile
NF2 = NS // 2  # 4096 fold2 pooled columns per core
TOPK = 48  # pooled columns kept per query (host-side, global)

_FP8 = ml_dtypes.float8_e4m3


def build_nc(b=B, ns=NS):
    """Build the per-core Bass program (SPMD: same program, per-core inputs)."""
    import concourse.tile as tile
    from concourse import bacc, mybir

    mt = b // 128
    nt = ns // NCHUNK

    nc = bacc.Bacc(None, target_bir_lowering=False)
    # [p, m, t, i, q] = xTpad[256t+128i+p, 128m+q]  (fp8, K padded to 512)
    xT8 = nc.dram_tensor(
        "xT8", [128, mt, 2, 2, 128], mybir.dt.float8e4, kind="ExternalInput"
    )
    # [p, pair, t, i, w] = XTpad[256t+128i+p, 1024*pair+w]  (chunk pair blocks)
    XT8 = nc.dram_tensor(
        "XT8", [128, nt // 2, 2, 2, 2 * NCHUNK], mybir.dt.float8e4, kind="ExternalInput"
    )
    # pooled[q, q4*1024 + s*512 + n] = fold2 of chunks {4*q4+s, 4*q4+2+s}
    pooled = nc.dram_tensor(
        "pooled", [b, NVT * 2 * NCHUNK], mybir.dt.float16, kind="ExternalOutput"
    )

    MAX = mybir.AluOpType.max

    # TRN2 drain rules: GPSIMD can't touch PSUM; any op reads at most ONE
    # non-scalar PSUM operand (a single AP may still span 2 banks).  So per
    # quad of chunks {4q..4q+3}: ACT 2-wide-copies the even pair to SBUF
    # fp16, DVE folds the odd pair's 2-bank PSUM AP against the copy
    # (tensor_tensor max), and Pool does the all-SBUF 2-wide finals.

    with tile.TileContext(nc) as tc:
        with (
            tc.tile_pool(name="wpool", bufs=1) as wpool,
            tc.tile_pool(name="ppool", bufs=4, space="PSUM") as ppool,
            tc.tile_pool(name="lpool", bufs=2) as lpool,
            tc.tile_pool(name="opool", bufs=3) as opool,
        ):
            # Per-chunk tiles so each matmul waits only on ITS chunk's DMA
            # (subtile deps on one big tile made the first matmul wait for the
            # whole 4.2MB input stream).  Loads split across the SP and GpSimd
            # DGE queues so per-dma issue cost doesn't serialize the startup.
            xT_m0 = wpool.tile([128, 2, 2, 128], mybir.dt.float8e4, name="xT_m0")
            xT_r = wpool.tile([128, mt - 1, 2, 2, 128], mybir.dt.float8e4, name="xT_r")
            nc.sync.dma_start(xT_m0[:], xT8[:, 0])
            XT_p = []
            for p in range(nt // 2):
                xtp = wpool.tile(
                    [128, 2, 2, 2 * NCHUNK], mybir.dt.float8e4, name="xtp", tag=f"xt{p}"
                )
                XT_p.append(xtp)
                if p % 2 == 0:
                    nc.sync.dma_start(xtp[:], XT8[:, p])
                else:
                    nc.gpsimd.dma_start(out=xtp[:], in_=XT8[:, p])
            # xT for m>=1 is not needed until ~5us in; load it after the
            # first chunk pairs so it doesn't hog the DMA engines at startup
            nc.gpsimd.dma_start(out=xT_r[:], in_=xT8[:, 1:])

            def xT_ap(m, t):
                return xT_m0[:, t] if m == 0 else xT_r[:, m - 1, t]

            for m in range(mt):
                # F[p, q, s*512+n]: fold2 of chunks {4q+s, 4q+2+s}; shipped as-is
                F = lpool.tile([128, NVT, 2 * NCHUNK], mybir.dt.float16, name="F")
                for h in range(2):  # half h = pairs 4h..4h+3 = 8 psum banks
                    pts = [
                        ppool.tile(
                            [128, 2, NCHUNK], mybir.dt.float32, name="pt", tag="pt"
                        )
                        for _ in range(4)
                    ]
                    # t-outer: 8 consecutive matmuls share the stationary
                    # operand (4 weight switches per m-tile).
                    for t in range(2):
                        for pr in range(4):
                            gp = 4 * h + pr  # global pair = chunks (2gp, 2gp+1)
                            for s in range(2):
                                nc.tensor.matmul(
                                    pts[pr][:, s],
                                    xT_ap(m, t),
                                    XT_p[gp][:, t, :, s * NCHUNK : (s + 1) * NCHUNK],
                                    start=(t == 0),
                                    stop=(t == 1),
                                    perf_mode=mybir.MatmulPerfMode.DoubleRow,
                                )
                    for j in range(2):  # quad q = 2h+j: pairs (2j, 2j+1) of half
                        q = 2 * h + j
                        c2 = lpool.tile(
                            [128, 2, NCHUNK],
                            mybir.dt.float16,
                            name="c2",
                            tag=f"c2{j}",
                        )
                        nc.scalar.copy(c2[:], pts[2 * j][:])
                        nc.vector.tensor_tensor(F[:, q], pts[2 * j + 1][:], c2[:], op=MAX)
                for half in range(2):
                    nc.sync.dma_start(
                        pooled[
                            m * 128 : (m + 1) * 128,
                            half * 2 * 2 * NCHUNK : (half + 1) * 2 * 2 * NCHUNK,
                        ],
                        F[:, 2 * half : 2 * half + 2].rearrange("p q w -> p (q w)"),
                    )
    nc.finalize()
    return nc


_NC = None


def _get_nc():
    global _NC
    if _NC is None:
        _NC = build_nc()
    return _NC


def _shard_perm(tt, ns):
    """Device row (FOLD*v+u)*512+n holds tt-sorted rank (v*512+n)*FOLD+u, so
    the 4 rows folded into pooled column (v,n) are adjacent in tt order."""
    order = np.argsort(tt, kind="stable")
    r = np.arange(ns)
    j, col = r // NCHUNK, r % NCHUNK
    v, u = j // FOLD, j % FOLD
    return order[(v * NCHUNK + col) * FOLD + u]


def _prep_in_maps(xf, X_train):
    xTpad = np.zeros((DPAD, B), np.float32)
    xTpad[:D] = xf.T
    xT8 = np.ascontiguousarray(
        xTpad.astype(_FP8).reshape(2, 2, 128, MT, 128).transpose(2, 3, 0, 1, 4)
    )
    in_maps = []
    perms = []
    biases = []
    for c in range(NCORES):
        Xs = X_train[c * NS : (c + 1) * NS]
        tt = (Xs.astype(np.float64) ** 2).sum(axis=1)
        perm = _shard_perm(tt, NS)
        perms.append(perm)
        XTpad = np.zeros((DPAD, NS), np.float32)
        XTpad[:D] = Xs[perm].T
        XT8 = np.ascontiguousarray(
            XTpad.astype(_FP8)
            .reshape(2, 2, 128, NT // 2, 2 * NCHUNK)
            .transpose(2, 3, 0, 1, 4)
        )
        tt_dev = tt[perm]
        # pooled col (q*1024 + s*512 + n) folds device rows
        # {(4q+s)*512+n, (4q+2+s)*512+n}; bias = mean tt/2 of the two
        td = tt_dev.reshape(NVT, 2, 2, NCHUNK)  # [q, k, s, n]
        bias = td.mean(axis=1).reshape(NF2) * 0.5  # [q, s, n] -> flat
        biases.append(bias.astype(np.float32))
        in_maps.append({"xT8": xT8, "XT8": XT8})
    return in_maps, perms, biases


def _refine(xf, X_train, Y_train, cand):
    """cand: [B, C] global candidate row indices (int64, may repeat)."""
    b = cand.shape[0]
    cand = np.sort(cand, axis=1)
    best = np.empty(b, dtype=np.int64)
    xd = xf.astype(np.float64)
    step = 128
    for s in range(0, b, step):
        e = min(s + step, b)
        Xc = X_train[cand[s:e]].astype(np.float64)  # [q, C, D]
        diff = xd[s:e, None, :] - Xc
        d2 = np.einsum("qcd,qcd->qc", diff, diff)
        best[s:e] = cand[s:e][np.arange(e - s), np.argmin(d2, axis=1)]
    return Y_train[best].astype(np.float32)


def kernel(x, X_train, Y_train, _trace=False, _tmpdir=None):
    from concourse.bass_utils import run_bass_kernel_spmd

    x = np.asarray(x, dtype=np.float32)
    X_train = np.asarray(X_train, dtype=np.float32)
    Y_train = np.asarray(Y_train, dtype=np.float32)
    xf = x.reshape(B, D)

    in_maps, perms, biases = _prep_in_maps(xf, X_train)
    nc = _get_nc()
    kw = {}
    if _trace:
        kw = {"trace": True, "tmpdir": _tmpdir}
    res = run_bass_kernel_spmd(nc, in_maps, core_ids=list(range(NCORES)), **kw)

    # scores[q, c*NF2 + col] = pooled - bias
    scores = np.empty((B, NCORES * NF2), np.float32)
    for c in range(NCORES):
        scores[:, c * NF2 : (c + 1) * NF2] = (
            res.results[c]["pooled"].astype(np.float32) - biases[c][None, :]
        )
    top = np.argpartition(-scores, TOPK, axis=1)[:, :TOPK]  # [B, K] pooled cols
    core = top // NF2
    col = top % NF2
    q4, rem = col // (2 * NCHUNK), col % (2 * NCHUNK)
    s, n = rem // NCHUNK, rem % NCHUNK
    # pooled col (q4,s,n) covers device rows (4q4+s)*512+n and (4q4+2+s)*512+n
    base = (4 * q4 + s) * NCHUNK + n
    devrows = np.stack([base, base + 2 * NCHUNK], axis=2)  # [B, K, 2]
    permtab = np.stack(perms)  # [NCORES, NS]
    cand = permtab[core[:, :, None], devrows] + core[:, :, None] * NS
    cand = cand.reshape(B, TOPK * 2)
    out = _refine(xf, X_train, Y_train, cand)
    if _trace:
        return out, res
    return out
